# revision 1
# baseline (speedup 1.0000x reference)
import sys

for _p in ("/opt/trn_rl_repo", "/root/.axon_site/_ro/trn_rl_repo"):
    if _p not in sys.path:
        sys.path.insert(0, _p)

import numpy as np
import ml_dtypes

from concourse import bass, bacc, mybir
from concourse.tile import TileContext
from concourse.bass_utils import run_bass_kernel_spmd

BF16 = ml_dtypes.bfloat16

# ---- problem constants (hardcoded per contract) ----
B, T, NB, D = 8, 2048, 22, 128
WIDTH = 64
FREQ = 1025
N_FFT = 2048
HOP = 512
STARTS = [0, 48, 96, 144, 192, 240, 288, 336, 384, 432, 480, 528,
          576, 624, 672, 720, 768, 816, 864, 912, 960, 961]
NCHUNK = 8          # full 128-bin frequency chunks (bins 0..1023); bin 1024 handled separately
TT = 256            # time-tile width
NTT = T // TT       # 8 time tiles
OUTC = 2047         # output columns: out[512*c + r], c in [0, 2047), r in [0,512)
N_CORES = 8

# band->chunk incidence: for chunk k, list of band indices overlapping bins [128k, 128k+128)
def _incidence():
    inc = []
    for k in range(NCHUNK):
        lo_bin, hi_bin = 128 * k, 128 * k + 128
        bands = []
        for n, s in enumerate(STARTS):
            lo, hi = max(lo_bin, s), min(hi_bin, s + WIDTH)
            if lo < hi and not (n == 21 and lo_bin <= 1024 < hi_bin):
                bands.append(n)
        inc.append(bands)
    return inc

INC = _incidence()
# packed weight-block column offsets: blocks ordered (k, comp, band)
def _wblock_layout():
    off = 0
    layout = {}  # (k, comp, n) -> col offset (128 cols each)
    for k in range(NCHUNK):
        for comp in range(2):
            for n in INC[k]:
                layout[(k, comp, n)] = off
                off += 128
    # bin-1024 blocks: band 21, w=63; 1 col each for comp 0/1
    layout[(8, 0, 21)] = off
    layout[(8, 1, 21)] = off + 1
    off += 2
    # bias rows (read from partition 0 as K=1 lhsT): one block per (k, comp)
    for k in range(NCHUNK + 1):
        for comp in range(2):
            layout[("bias", k, comp)] = off
            off += 128 if k < NCHUNK else 1
    return layout, off

WLAYOUT, WCOLS = _wblock_layout()

_CACHE = {}


def _build_nc():
    f32 = mybir.dt.float32
    bf16 = mybir.dt.bfloat16
    nc = bacc.Bacc(None, target_bir_lowering=False, debug=False)

    zp = nc.dram_tensor("zp", [128, NB, T], bf16, kind="ExternalInput")
    mixp = nc.dram_tensor("mixp", [128, NCHUNK + 1, 2, T], bf16, kind="ExternalInput")
    mp = nc.dram_tensor("mp", [128, NCHUNK + 1, 2, FREQ], bf16, kind="ExternalInput")
    wb = nc.dram_tensor("wb", [128, WCOLS], bf16, kind="ExternalInput")
    edge = nc.dram_tensor("edge", [128, 4, 2], f32, kind="ExternalInput")
    winv_d = nc.dram_tensor("winv", [128, 16], f32, kind="ExternalInput")
    winrev_d = nc.dram_tensor("winrev", [128, 8], f32, kind="ExternalInput")
    jrev_d = nc.dram_tensor("jrev", [128, 128], bf16, kind="ExternalInput")
    e00_d = nc.dram_tensor("e00", [1, 128], bf16, kind="ExternalInput")
    outp = nc.dram_tensor("outp", [4, 128, OUTC], f32, kind="ExternalOutput")

    with TileContext(nc) as tc:
        with (
            tc.tile_pool(name="singles", bufs=1) as singles,
            tc.tile_pool(name="zpool", bufs=2) as zpool,
            tc.tile_pool(name="mixpool", bufs=2) as mixpool,
            tc.tile_pool(name="spec", bufs=40) as specpool,
            tc.tile_pool(name="mrmi", bufs=9) as mrmipool,
            tc.tile_pool(name="tmp", bufs=8) as tmppool,
            tc.tile_pool(name="fpool", bufs=3) as fpool,
            tc.tile_pool(name="fplus", bufs=11) as fpluspool,
            tc.tile_pool(name="maskps", bufs=3, space="PSUM") as maskpool,
            tc.tile_pool(name="dftps", bufs=3, space="PSUM") as dftpool,
            tc.tile_pool(name="revps", bufs=2, space="PSUM") as revpool,
        ):
            mp_t = singles.tile([128, NCHUNK + 1, 2, FREQ], bf16, tag="mp")
            nc.sync.dma_start(mp_t[:], mp[:])
            wb_t = singles.tile([128, WCOLS], bf16, tag="wb")
            nc.sync.dma_start(wb_t[:], wb[:])
            ones_t = singles.tile([1, TT], bf16, tag="ones")
            nc.vector.memset(ones_t[:], 1.0)
            edge_t = singles.tile([128, 4, 2], f32, tag="edge")
            nc.sync.dma_start(edge_t[:], edge[:])
            winv_t = singles.tile([128, 16], f32, tag="winv")
            nc.sync.dma_start(winv_t[:], winv_d[:])
            winrev_t = singles.tile([128, 8], f32, tag="winrev")
            nc.sync.dma_start(winrev_t[:], winrev_d[:])
            jrev_t = singles.tile([128, 128], bf16, tag="jrev")
            nc.sync.dma_start(jrev_t[:], jrev_d[:])
            e00_t = singles.tile([1, 128], bf16, tag="e00")
            nc.sync.dma_start(e00_t[:], e00_d[:])

            outs = []
            for u in range(4):
                o = singles.tile([128, OUTC], f32, tag=f"out{u}")
                nc.vector.memset(o[:], 0.0)
                outs.append(o)

            for tau in range(NTT):
                t0 = tau * TT
                ztile = zpool.tile([128, NB, TT], bf16, tag="z")
                nc.sync.dma_start(ztile[:], zp[:, :, t0:t0 + TT])
                mtile = mixpool.tile([128, NCHUNK + 1, 2, TT], bf16, tag="mix")
                nc.sync.dma_start(mtile[:], mixp[:, :, :, t0:t0 + TT])

                spec_r, spec_i = [], []
                for k in range(NCHUNK + 1):
                    npart = 128 if k < NCHUNK else 1
                    bands = INC[k] if k < NCHUNK else [21]
                    ps_pair = []
                    for comp in range(2):
                        ps = maskpool.tile([npart, TT], f32, tag="maskps")
                        for bi, n in enumerate(bands):
                            coloff = WLAYOUT[(k, comp, n)]
                            nc.tensor.matmul(
                                ps[:npart, :],
                                wb_t[:, coloff:coloff + npart],
                                ztile[:, n, :],
                                start=(bi == 0),
                                stop=False,
                            )
                        boff = WLAYOUT[("bias", k, comp)]
                        nc.tensor.matmul(
                            ps[:npart, :],
                            wb_t[0:1, boff:boff + npart],
                            ones_t[:],
                            start=False,
                            stop=True,
                        )
                        ps_pair.append(ps)
                    ps_r, ps_i = ps_pair
                    mr = mrmipool.tile([npart, TT], bf16, tag="mrmi")
                    nc.scalar.copy(mr[:npart, :], ps_r[:npart, :])
                    if k < NCHUNK:
                        mi = mrmipool.tile([npart, TT], bf16, tag="mrmi")
                        nc.scalar.copy(mi[:npart, :], ps_i[:npart, :])
                    min_ = mrmipool.tile([npart, TT], bf16, tag="mrmi")
                    nc.scalar.activation(min_[:npart, :], ps_i[:npart, :],
                                         mybir.ActivationFunctionType.Copy,
                                         scale=-1.0)
                    mxr = mtile[:npart, k, 0, :]
                    mxi = mtile[:npart, k, 1, :]
                    p1 = tmppool.tile([npart, TT], bf16, tag="tmp")
                    nc.vector.tensor_mul(p1[:npart, :], mr[:npart, :], mxr)
                    p2 = tmppool.tile([npart, TT], bf16, tag="tmp")
                    nc.vector.tensor_mul(p2[:npart, :], min_[:npart, :], mxi)
                    sr = specpool.tile([npart, TT], bf16, tag="spec")
                    nc.gpsimd.tensor_add(sr[:npart, :], p1[:npart, :], p2[:npart, :])
                    spec_r.append(sr)
                    if k < NCHUNK:
                        p3 = tmppool.tile([npart, TT], bf16, tag="tmp")
                        nc.vector.tensor_mul(p3[:npart, :], mr[:npart, :], mxi)
                        p4 = tmppool.tile([npart, TT], bf16, tag="tmp")
                        nc.vector.tensor_mul(p4[:npart, :], mi[:npart, :], mxr)
                        si = specpool.tile([npart, TT], bf16, tag="spec")
                        nc.gpsimd.tensor_add(si[:npart, :], p3[:npart, :], p4[:npart, :])
                        spec_i.append(si)
                    else:
                        spec_i.append(None)

                def ola_range(q):
                    a = t0 + q - 2
                    fa = 0
                    if a < 0:
                        fa = -a
                        a = 0
                    b_ = t0 + q - 2 + TT
                    fb = TT
                    if b_ > OUTC:
                        fb = TT - (b_ - OUTC)
                        b_ = OUTC
                    return a, b_, fa, fb

                # s = 1024 singleton: frames[1024] = P[1024] (Q[1024]=0, win=1)
                ps1024 = dftpool.tile([1, TT], f32, tag="dftps")
                for k in range(NCHUNK + 1):
                    kp = 128 if k < NCHUNK else 1
                    nc.tensor.matmul(ps1024[:1, :], mp_t[:kp, k, 0, 1024:1025],
                                     spec_r[k][:kp, :],
                                     start=(k == 0), stop=(k == NCHUNK))
                f1024 = fpluspool.tile([1, TT], bf16, tag="fplus")
                nc.scalar.copy(f1024[:1, :], ps1024[:1, :])

                fplus = []
                for blk in range(8):
                    soff = 128 * blk
                    Pps = dftpool.tile([128, TT], f32, tag="dftps")
                    for k in range(NCHUNK + 1):
                        kp = 128 if k < NCHUNK else 1
                        nc.tensor.matmul(Pps[:], mp_t[:kp, k, 0, soff:soff + 128],
                                         spec_r[k][:kp, :],
                                         start=(k == 0), stop=(k == NCHUNK))
                    Qps = dftpool.tile([128, TT], f32, tag="dftps")
                    for k in range(NCHUNK):
                        nc.tensor.matmul(Qps[:], mp_t[:, k, 1, soff:soff + 128],
                                         spec_i[k][:],
                                         start=(k == 0), stop=(k == NCHUNK - 1))
                    qs = fpool.tile([128, TT], f32, tag="qs")
                    nc.scalar.copy(qs[:], Qps[:])
                    # direct half: OUT += (P - Q) * win[s]
                    fm = fpool.tile([128, TT], f32, tag="fm")
                    nc.vector.tensor_sub(fm[:], Pps[:], qs[:])
                    q_, u_ = blk // 4, blk % 4
                    a, b_, fa, fb = ola_range(q_)
                    if fb > fa:
                        o = outs[u_]
                        nc.vector.scalar_tensor_tensor(
                            o[:, a:b_], fm[:, fa:fb], winv_t[:, blk:blk + 1],
                            o[:, a:b_], mybir.AluOpType.mult, mybir.AluOpType.add)
                    # mirrored half source: (P + Q) * win[2048-s] -> bf16
                    fp = fpool.tile([128, TT], f32, tag="fp")
                    nc.vector.tensor_add(fp[:], Pps[:], qs[:])
                    fpw = fpluspool.tile([128, TT], bf16, tag="fplus")
                    nc.scalar.activation(fpw[:], fp[:],
                                         mybir.ActivationFunctionType.Identity,
                                         scale=winrev_t[:, blk:blk + 1])
                    fplus.append(fpw)

                # mirrored half: OLA blocks j=8..15 via partition reversal on PE
                for j in range(8, 16):
                    Frev = revpool.tile([128, TT], f32, tag="revps")
                    nc.tensor.matmul(Frev[:], jrev_t[:], fplus[15 - j][:],
                                     start=True, stop=False)
                    bsrc = f1024 if j == 8 else fplus[16 - j]
                    nc.tensor.matmul(Frev[:], e00_t[:1, :], bsrc[:1, :],
                                     start=False, stop=True)
                    q_, u_ = j // 4, j % 4
                    a, b_, fa, fb = ola_range(q_)
                    if fb > fa:
                        o = outs[u_]
                        nc.vector.tensor_add(o[:, a:b_], o[:, a:b_], Frev[:, fa:fb])

            # env edge fixup: columns c=0 (missing q=3 frame) and c=2046 (missing q=0)
            for u in range(4):
                for j, c in ((0, 0), (1, OUTC - 1)):
                    nc.vector.tensor_mul(outs[u][:, c:c + 1], outs[u][:, c:c + 1],
                                         edge_t[:, u, j:j + 1])
            for u in range(4):
                nc.sync.dma_start(outp[u], outs[u][:])

    if not nc.is_finalized():
        nc.finalize()
    return nc


def _host_constants():
    # overlap counts per frequency bin
    wgt = np.zeros(FREQ, np.float64)
    for n, s in enumerate(STARTS):
        wgt[s:s + WIDTH] += 1.0
    wgt = np.maximum(wgt, 1.0)

    # scaled IDFT basis, s in [0, 1025), window NOT folded (applied via ACT scale)
    # frames[s] = P[s] - Q[s];  frames[2048-s] = P[s] + Q[s]
    s_idx = np.arange(N_FFT)
    win = 0.5 * (1.0 - np.cos(2.0 * np.pi * s_idx / N_FFT))
    f_idx = np.arange(FREQ)
    c_f = np.full(FREQ, 2.0)
    c_f[0] = 1.0
    c_f[N_FFT // 2] = 1.0
    sh = np.arange(FREQ)  # s in [0, 1025)
    ang = 2.0 * np.pi * np.outer(f_idx, sh) / N_FFT
    scale = (c_f / (N_FFT * 1.5))[:, None]
    Mc = np.cos(ang) * scale
    Ms = np.sin(ang) * scale
    mp = np.zeros((128, NCHUNK + 1, 2, FREQ), np.float64)
    for k in range(NCHUNK):
        mp[:, k, 0, :] = Mc[128 * k:128 * k + 128]
        mp[:, k, 1, :] = Ms[128 * k:128 * k + 128]
    mp[0, 8, 0, :] = Mc[1024]
    mp[0, 8, 1, :] = Ms[1024]
    mp_bf = mp.astype(BF16)
    # window scale vectors (f32): winv[p, b] = win[128b+p] for the direct half;
    # winrev[p, b] = win[(2048 - (128b+p)) % 2048] for the mirrored half
    winv = np.zeros((128, 16), np.float32)
    winrev = np.zeros((128, 8), np.float32)
    for bb in range(16):
        winv[:, bb] = win[128 * bb + np.arange(128)]
    for bb in range(8):
        winrev[:, bb] = win[(2048 - (128 * bb + np.arange(128))) % 2048]
    # partition-reversal permutation: out[p'] = in[128-p'] for p' in [1,128)
    jrev = np.zeros((128, 128), np.float64)
    for p in range(1, 128):
        jrev[p, 128 - p] = 1.0
    jrev_bf = jrev.astype(BF16)
    e00 = np.zeros((1, 128), np.float64)
    e00[0, 0] = 1.0
    e00_bf = e00.astype(BF16)

    # edge ratios for the two output columns with only 3 overlapping frames
    w2 = win * win
    env0 = w2[np.arange(512)] + w2[512 + np.arange(512)] + w2[1024 + np.arange(512)]
    envL = w2[512 + np.arange(512)] + w2[1024 + np.arange(512)] + w2[1536 + np.arange(512)]
    edge = np.zeros((128, 4, 2), np.float32)
    for u in range(4):
        r = 128 * u + np.arange(128)
        edge[:, u, 0] = (1.5 / env0[r]).astype(np.float32)
        edge[:, u, 1] = (1.5 / envL[r]).astype(np.float32)
    return wgt, mp_bf, edge, winv, winrev, jrev_bf, e00_bf


def _pack_weights(W, b, wgt):
    # de-interleave + fold 1/wgt:  W2[n,d,w] (real), W2[n,d,64+w] (imag)
    W = np.asarray(W, np.float64)
    b = np.asarray(b, np.float64)
    W2 = np.zeros((NB, D, 128), np.float64)
    for n, s in enumerate(STARTS):
        g = wgt[s:s + WIDTH]
        W2[n, :, :WIDTH] = W[n, :, 0::2] / g[None, :]
        W2[n, :, WIDTH:] = W[n, :, 1::2] / g[None, :]
    wbp = np.zeros((128, WCOLS), np.float64)
    for key, off in WLAYOUT.items():
        if key[0] == "bias":
            continue
        k, comp, n = key
        s = STARTS[n]
        if k < NCHUNK:
            blk = np.zeros((D, 128), np.float64)
            for j in range(128):
                w = 128 * k + j - s
                if 0 <= w < WIDTH:
                    blk[:, j] = W2[n, :, comp * WIDTH + w]
            wbp[:, off:off + 128] = blk
        else:
            wbp[:, off] = W2[n, :, comp * WIDTH + 63]
    # bias rows on partition 0
    bias_f = np.zeros((FREQ, 2), np.float64)
    for f in range(FREQ):
        for n, s in enumerate(STARTS):
            w = f - s
            if 0 <= w < WIDTH:
                bias_f[f, 0] += b[n, 2 * w]
                bias_f[f, 1] += b[n, 2 * w + 1]
        bias_f[f] /= wgt[f]
    for k in range(NCHUNK + 1):
        for comp in range(2):
            off = WLAYOUT[("bias", k, comp)]
            if k < NCHUNK:
                wbp[0, off:off + 128] = bias_f[128 * k:128 * k + 128, comp]
            else:
                wbp[0, off] = bias_f[1024, comp]
    return wbp.astype(BF16)


def kernel(z, mix_spec, W, b):
    if "nc" not in _CACHE:
        _CACHE["nc"] = _build_nc()
        _CACHE["consts"] = _host_constants()
    nc = _CACHE["nc"]
    wgt, mp_bf, edge, winv, winrev, jrev_bf, e00_bf = _CACHE["consts"]
    wbp = _pack_weights(W, b, wgt)

    in_maps = []
    for core in range(N_CORES):
        zb = np.ascontiguousarray(np.transpose(z[core], (2, 1, 0))).astype(BF16)
        mixp = np.zeros((128, NCHUNK + 1, 2, T), BF16)
        mx = mix_spec[core]  # (2, T, FREQ)
        mxT = np.transpose(mx, (0, 2, 1))  # (2, FREQ, T)
        for k in range(NCHUNK):
            mixp[:, k, 0, :] = mxT[0, 128 * k:128 * k + 128].astype(BF16)
            mixp[:, k, 1, :] = mxT[1, 128 * k:128 * k + 128].astype(BF16)
        mixp[0, 8, 0, :] = mxT[0, 1024].astype(BF16)
        mixp[0, 8, 1, :] = mxT[1, 1024].astype(BF16)
        in_maps.append({
            "zp": zb,
            "mixp": mixp,
            "mp": mp_bf,
            "wb": wbp,
            "edge": edge,
            "winv": winv,
            "winrev": winrev,
            "jrev": jrev_bf,
            "e00": e00_bf,
        })

    res = run_bass_kernel_spmd(nc, in_maps, core_ids=list(range(N_CORES)))
    out = np.empty((B, HOP * (T - 1)), np.float32)
    for core in range(N_CORES):
        o = res.results[core]["outp"]  # (4, 128, OUTC)
        out[core] = np.ascontiguousarray(np.transpose(o, (2, 0, 1))).reshape(-1)
    return out



# revision 11
# speedup vs baseline: 1.3580x; 1.3580x over previous
import sys

for _p in ("/opt/trn_rl_repo", "/root/.axon_site/_ro/trn_rl_repo"):
    if _p not in sys.path:
        sys.path.insert(0, _p)

import numpy as np
import ml_dtypes

from concourse import bass, bacc, mybir
from concourse.tile import TileContext
from concourse.bass_utils import run_bass_kernel_spmd

BF16 = ml_dtypes.bfloat16

# ---- problem constants (hardcoded per contract) ----
B, T, NB, D = 8, 2048, 22, 128
WIDTH = 64
FREQ = 1025
N_FFT = 2048
HOP = 512
STARTS = [0, 48, 96, 144, 192, 240, 288, 336, 384, 432, 480, 528,
          576, 624, 672, 720, 768, 816, 864, 912, 960, 961]
NCHUNK = 8          # full 128-bin frequency chunks (bins 0..1023); bin 1024 = chunk 8
TT = 512            # time-tile width
NTT = T // TT       # 4 time tiles
OUTC = 2047         # output columns: out[512*c + r], c in [0, 2047), r in [0,512)
N_CORES = 8

# band->chunk incidence: for chunk k, list of band indices overlapping bins [128k, 128k+128)
def _incidence():
    inc = []
    for k in range(NCHUNK):
        lo_bin, hi_bin = 128 * k, 128 * k + 128
        bands = []
        for n, s in enumerate(STARTS):
            lo, hi = max(lo_bin, s), min(hi_bin, s + WIDTH)
            if lo < hi and not (n == 21 and lo_bin <= 1024 < hi_bin):
                bands.append(n)
        inc.append(bands)
    return inc

INC = _incidence()

# packed weight-block column offsets: blocks ordered (k, comp, band), 128 cols each;
# then the two 1-col blocks for bin 1024 (band 21, w=63)
def _wblock_layout():
    off = 0
    layout = {}
    for k in range(NCHUNK):
        for comp in range(2):
            for n in INC[k]:
                layout[(k, comp, n)] = off
                off += 128
    layout[(8, 0, 21)] = off
    layout[(8, 1, 21)] = off + 1
    off += 2
    return layout, off

WLAYOUT, WCOLS = _wblock_layout()

_CACHE = {}


def _build_nc():
    f32 = mybir.dt.float32
    bf16 = mybir.dt.bfloat16
    AL = mybir.AluOpType
    nc = bacc.Bacc(None, target_bir_lowering=False, debug=False)

    zp = nc.dram_tensor("zp", [128, NB, T], bf16, kind="ExternalInput")
    mixp = nc.dram_tensor("mixp", [128, NCHUNK + 1, 2, T], bf16, kind="ExternalInput")
    mpc_d = nc.dram_tensor("mpc", [128, NCHUNK + 1, FREQ], bf16, kind="ExternalInput")
    mps_d = nc.dram_tensor("mps", [128, NCHUNK, FREQ - 1], bf16, kind="ExternalInput")
    wb = nc.dram_tensor("wb", [128, WCOLS], bf16, kind="ExternalInput")
    biasb_d = nc.dram_tensor("biasb", [128, NCHUNK + 1, 2], f32, kind="ExternalInput")
    edge_d = nc.dram_tensor("edge", [128, 4, 2], f32, kind="ExternalInput")
    jrev_d = nc.dram_tensor("jrev", [128, 128], bf16, kind="ExternalInput")
    outp = nc.dram_tensor("outp", [4, 128, OUTC], f32, kind="ExternalOutput")

    def ola_range(t0, delta):
        a = t0 + delta
        fa = 0
        if a < 0:
            fa = -a
            a = 0
        b_ = t0 + delta + TT
        fb = TT
        if b_ > OUTC:
            fb = TT - (b_ - OUTC)
            b_ = OUTC
        return a, b_, fa, fb

    with TileContext(nc) as tc:
        with (
            tc.tile_pool(name="singles", bufs=1) as singles,
            tc.tile_pool(name="zpool", bufs=9) as zpool,
            tc.tile_pool(name="mixpool", bufs=6) as mixpool,
            tc.tile_pool(name="spec", bufs=33) as specpool,
            tc.tile_pool(name="mrmi", bufs=4) as mrmipool,
            tc.tile_pool(name="ppool", bufs=6) as ppool,
            tc.tile_pool(name="fpool", bufs=4) as fpool,
            tc.tile_pool(name="revbf", bufs=1) as revbfpool,
            tc.tile_pool(name="maskps", bufs=3, space="PSUM") as maskpool,
            tc.tile_pool(name="dftps", bufs=3, space="PSUM") as dftpool,
            tc.tile_pool(name="endps", bufs=2, space="PSUM") as endpool,
        ):
            mpc_t = singles.tile([128, NCHUNK + 1, FREQ], bf16, tag="mpc")
            nc.sync.dma_start(mpc_t[:], mpc_d[:])
            mps_t = singles.tile([128, NCHUNK, FREQ - 1], bf16, tag="mps")
            nc.sync.dma_start(mps_t[:], mps_d[:])
            wb_t = singles.tile([128, WCOLS], bf16, tag="wb")
            nc.sync.dma_start(wb_t[:], wb[:])
            biasb_t = singles.tile([128, NCHUNK + 1, 2], f32, tag="biasb")
            nc.sync.dma_start(biasb_t[:], biasb_d[:])
            edge_t = singles.tile([128, 4, 2], f32, tag="edge")
            nc.sync.dma_start(edge_t[:], edge_d[:])
            jrev_t = singles.tile([128, 128], bf16, tag="jrev")
            nc.sync.dma_start(jrev_t[:], jrev_d[:])

            outs, outs_rev = [], []
            for u in range(4):
                o = singles.tile([128, OUTC], f32, tag=f"out{u}")
                nc.vector.memset(o[:], 0.0)
                outs.append(o)
                orv = singles.tile([128, OUTC], f32, tag=f"outrev{u}")
                nc.gpsimd.memset(orv[:], 0.0)
                outs_rev.append(orv)
            rowacc = singles.tile([1, OUTC], f32, tag="rowacc")
            nc.vector.memset(rowacc[:], 0.0)

            def idft_stage(tau, sr, si):
                t0 = tau * TT
                # frames[1024] accumulation into rowacc (out row r=0, col c=t)
                ps1 = dftpool.tile([1, TT], f32, tag="dftps")
                for k in range(NCHUNK + 1):
                    kp = 128 if k < NCHUNK else 1
                    nc.tensor.matmul(ps1[:1, :], mpc_t[:kp, k, 1024:1025],
                                     sr[k][:kp, :],
                                     start=(k == 0), stop=(k == NCHUNK))
                wr = min(TT, OUTC - t0)
                nc.vector.tensor_add(rowacc[0:1, t0:t0 + wr],
                                     rowacc[0:1, t0:t0 + wr], ps1[0:1, :wr])
                for blk in range(8):
                    soff = 128 * blk
                    Pps = dftpool.tile([128, TT], f32, tag="dftps")
                    for k in range(NCHUNK + 1):
                        kp = 128 if k < NCHUNK else 1
                        nc.tensor.matmul(Pps[:], mpc_t[:kp, k, soff:soff + 128],
                                         sr[k][:kp, :],
                                         start=(k == 0), stop=(k == NCHUNK))
                    Qps = dftpool.tile([128, TT], f32, tag="dftps")
                    for k in range(NCHUNK):
                        nc.tensor.matmul(Qps[:], mps_t[:, k, soff:soff + 128],
                                         si[k][:],
                                         start=(k == 0), stop=(k == NCHUNK - 1))
                    qs2 = fpool.tile([128, TT], f32, tag="qs", bufs=2)
                    nc.scalar.activation(qs2[:], Qps[:],
                                         mybir.ActivationFunctionType.Identity,
                                         scale=2.0)
                    # window is folded into the basis: P/Q are pre-windowed
                    fmw = fpool.tile([128, TT], f32, tag="f")
                    nc.vector.scalar_tensor_tensor(fmw[:], qs2[:], -0.5, Pps[:],
                                                   AL.mult, AL.add)
                    fpw = fpool.tile([128, TT], f32, tag="f")
                    nc.gpsimd.tensor_add(fpw[:], fmw[:], qs2[:])
                    # direct half: frames[s], s = 128*blk + p; col c = t + q - 2
                    q, u = blk // 4, blk % 4
                    a, b_, fa, fb = ola_range(t0, q - 2)
                    if fb > fa:
                        o = outs[u]
                        nc.vector.tensor_add(o[:, a:b_], o[:, a:b_],
                                             fmw[:, fa:fb])
                    # mirrored half into reversed-layout accumulator
                    qp = 1 + (1 if blk >= 4 else 0)
                    ur = 4 * qp - 1 - blk
                    a, b_, fa, fb = ola_range(t0, 2 - qp)
                    if fb > fa:
                        o = outs_rev[ur]
                        nc.gpsimd.tensor_add(o[:, a:b_], o[:, a:b_],
                                             fpw[:, fa:fb])

            prev = None
            for tau in range(NTT):
                t0 = tau * TT
                ztiles = []
                for n in range(NB):
                    zt = zpool.tile([128, TT], bf16, tag="z")
                    nc.sync.dma_start(zt[:], zp[:, n, t0:t0 + TT])
                    ztiles.append(zt)
                mtiles = []
                for k in range(NCHUNK + 1):
                    npart = 128 if k < NCHUNK else 1
                    mt = mixpool.tile([npart, 2, TT], bf16, tag="mix")
                    nc.sync.dma_start(mt[:], mixp[:npart, k, :, t0:t0 + TT])
                    mtiles.append(mt)

                sr_list, si_list = [], []
                for k in range(NCHUNK + 1):
                    npart = 128 if k < NCHUNK else 1
                    bands = INC[k] if k < NCHUNK else [21]
                    pspair = []
                    for comp in range(2):
                        ps = maskpool.tile([npart, TT], f32, tag="maskps")
                        for bi, n in enumerate(bands):
                            coloff = WLAYOUT[(k, comp, n)]
                            nc.tensor.matmul(
                                ps[:npart, :],
                                wb_t[:, coloff:coloff + npart],
                                ztiles[n][:],
                                start=(bi == 0),
                                stop=(bi == len(bands) - 1),
                            )
                        pspair.append(ps)
                    ps_r, ps_i = pspair
                    # PSUM -> bf16 with folded (normalized) bias
                    mr = mrmipool.tile([npart, TT], bf16, tag="mr")
                    nc.scalar.activation(mr[:npart, :], ps_r[:npart, :],
                                         mybir.ActivationFunctionType.Identity,
                                         bias=biasb_t[:npart, k, 0:1])
                    mi = mrmipool.tile([npart, TT], bf16, tag="mi")
                    nc.scalar.activation(mi[:npart, :], ps_i[:npart, :],
                                         mybir.ActivationFunctionType.Identity,
                                         bias=biasb_t[:npart, k, 1:2])
                    mxr = mtiles[k][:npart, 0, :]
                    mxi = mtiles[k][:npart, 1, :]
                    p1 = ppool.tile([npart, TT], bf16, tag="p")
                    nc.vector.tensor_mul(p1[:npart, :], mr[:npart, :], mxr)
                    p2 = ppool.tile([npart, TT], bf16, tag="p")
                    nc.vector.tensor_mul(p2[:npart, :], mi[:npart, :], mxi)
                    sr = specpool.tile([npart, TT], bf16, tag="spec")
                    nc.gpsimd.tensor_sub(sr[:npart, :], p1[:npart, :], p2[:npart, :])
                    sr_list.append(sr)
                    if k < NCHUNK:
                        p3 = ppool.tile([npart, TT], bf16, tag="p")
                        nc.vector.tensor_mul(p3[:npart, :], mr[:npart, :], mxi)
                        p4 = ppool.tile([npart, TT], bf16, tag="p")
                        nc.vector.tensor_mul(p4[:npart, :], mi[:npart, :], mxr)
                        si = specpool.tile([npart, TT], bf16, tag="spec")
                        nc.gpsimd.tensor_add(si[:npart, :], p3[:npart, :], p4[:npart, :])
                        si_list.append(si)

                if prev is not None:
                    idft_stage(prev[0], prev[1], prev[2])
                prev = (tau, sr_list, si_list)

            idft_stage(prev[0], prev[1], prev[2])

            # end stage: fold reversed accumulators into outs via one partition
            # reversal per 512-col block, then row-0 boundary terms
            for u in range(4):
                rb = revbfpool.tile([128, OUTC], bf16, tag="revbf")
                nc.scalar.copy(rb[:], outs_rev[u][:])
                for c4 in range(4):
                    lo = 512 * c4
                    hi = min(lo + 512, OUTC)
                    w = hi - lo
                    rv = endpool.tile([128, 512], f32, tag="revps")
                    nc.tensor.matmul(rv[:, :w], jrev_t[:], rb[:, lo:hi],
                                     start=True, stop=True)
                    nc.vector.tensor_add(outs[u][:, lo:hi], outs[u][:, lo:hi],
                                         rv[:, :w])
            for u in (1, 2, 3):
                nc.vector.tensor_add(outs[u][0:1, :], outs[u][0:1, :],
                                     outs_rev[u - 1][0:1, :])
            nc.vector.tensor_add(outs[0][0:1, 1:OUTC], outs[0][0:1, 1:OUTC],
                                 outs_rev[3][0:1, 0:OUTC - 1])
            nc.vector.tensor_add(outs[0][0:1, :], outs[0][0:1, :],
                                 rowacc[0:1, :])

            # env edge fixup: columns c=0 (missing q=3 frame) and c=2046 (missing q=0)
            for u in range(4):
                for j, c in ((0, 0), (1, OUTC - 1)):
                    nc.vector.tensor_mul(outs[u][:, c:c + 1], outs[u][:, c:c + 1],
                                         edge_t[:, u, j:j + 1])
            for u in range(4):
                nc.sync.dma_start(outp[u], outs[u][:])

    if not nc.is_finalized():
        nc.finalize()
    return nc


def _host_constants():
    # overlap counts per frequency bin
    wgt = np.zeros(FREQ, np.float64)
    for n, s in enumerate(STARTS):
        wgt[s:s + WIDTH] += 1.0
    wgt = np.maximum(wgt, 1.0)

    # scaled IDFT basis, s in [0, 1025), window applied afterwards via STT scale
    # frames[s] = P[s] - Q[s];  frames[2048-s] = P[s] + Q[s]
    s_idx = np.arange(N_FFT)
    win = 0.5 * (1.0 - np.cos(2.0 * np.pi * s_idx / N_FFT))
    f_idx = np.arange(FREQ)
    c_f = np.full(FREQ, 2.0)
    c_f[0] = 1.0
    c_f[N_FFT // 2] = 1.0
    sh = np.arange(FREQ)
    ang = 2.0 * np.pi * np.outer(f_idx, sh) / N_FFT
    scale = (c_f / (N_FFT * 1.5))[:, None]
    Mc = np.cos(ang) * scale * win[None, :FREQ]
    Ms = np.sin(ang) * scale * win[None, :FREQ]
    mpc = np.zeros((128, NCHUNK + 1, FREQ), np.float64)
    mps = np.zeros((128, NCHUNK, FREQ - 1), np.float64)
    for k in range(NCHUNK):
        mpc[:, k, :] = Mc[128 * k:128 * k + 128]
        mps[:, k, :] = Ms[128 * k:128 * k + 128, :FREQ - 1]
    mpc[0, 8, :] = Mc[1024]
    # partition-reversal permutation: out[p'] = in[128-p'] for p' in [1,128); row 0 -> 0
    jrev = np.zeros((128, 128), np.float64)
    for p in range(1, 128):
        jrev[p, 128 - p] = 1.0

    # edge ratios for the two output columns with only 3 overlapping frames
    w2 = win * win
    env0 = w2[np.arange(512)] + w2[512 + np.arange(512)] + w2[1024 + np.arange(512)]
    envL = w2[512 + np.arange(512)] + w2[1024 + np.arange(512)] + w2[1536 + np.arange(512)]
    edge = np.zeros((128, 4, 2), np.float32)
    for u in range(4):
        r = 128 * u + np.arange(128)
        edge[:, u, 0] = (1.5 / env0[r]).astype(np.float32)
        edge[:, u, 1] = (1.5 / envL[r]).astype(np.float32)
    return (wgt, mpc.astype(BF16), mps.astype(BF16), edge,
            jrev.astype(BF16))


def _pack_weights(W, b, wgt):
    # de-interleave + fold 1/wgt:  W2[n,d,w] (real), W2[n,d,64+w] (imag)
    W = np.asarray(W, np.float64)
    b = np.asarray(b, np.float64)
    W2 = np.zeros((NB, D, 128), np.float64)
    for n, s in enumerate(STARTS):
        g = wgt[s:s + WIDTH]
        W2[n, :, :WIDTH] = W[n, :, 0::2] / g[None, :]
        W2[n, :, WIDTH:] = W[n, :, 1::2] / g[None, :]
    wbp = np.zeros((128, WCOLS), np.float64)
    for key, off in WLAYOUT.items():
        k, comp, n = key
        s = STARTS[n]
        if k < NCHUNK:
            blk = np.zeros((D, 128), np.float64)
            for j in range(128):
                w = 128 * k + j - s
                if 0 <= w < WIDTH:
                    blk[:, j] = W2[n, :, comp * WIDTH + w]
            wbp[:, off:off + 128] = blk
        else:
            wbp[:, off] = W2[n, :, comp * WIDTH + 63]
    # per-bin normalized bias
    bias_f = np.zeros((FREQ, 2), np.float64)
    for f in range(FREQ):
        for n, s in enumerate(STARTS):
            w = f - s
            if 0 <= w < WIDTH:
                bias_f[f, 0] += b[n, 2 * w]
                bias_f[f, 1] += b[n, 2 * w + 1]
        bias_f[f] /= wgt[f]
    biasb = np.zeros((128, NCHUNK + 1, 2), np.float32)
    for k in range(NCHUNK):
        biasb[:, k, :] = bias_f[128 * k:128 * k + 128, :]
    biasb[0, 8, :] = bias_f[1024, :]
    return wbp.astype(BF16), biasb


def kernel(z, mix_spec, W, b):
    if "nc" not in _CACHE:
        _CACHE["nc"] = _build_nc()
        _CACHE["consts"] = _host_constants()
    nc = _CACHE["nc"]
    wgt, mpc_bf, mps_bf, edge, jrev_bf = _CACHE["consts"]
    wbp, biasb = _pack_weights(W, b, wgt)

    in_maps = []
    for core in range(N_CORES):
        zb = np.ascontiguousarray(np.transpose(z[core], (2, 1, 0))).astype(BF16)
        mixp = np.zeros((128, NCHUNK + 1, 2, T), BF16)
        mx = mix_spec[core]  # (2, T, FREQ)
        mxT = np.transpose(mx, (0, 2, 1))  # (2, FREQ, T)
        for k in range(NCHUNK):
            mixp[:, k, 0, :] = mxT[0, 128 * k:128 * k + 128].astype(BF16)
            mixp[:, k, 1, :] = mxT[1, 128 * k:128 * k + 128].astype(BF16)
        mixp[0, 8, 0, :] = mxT[0, 1024].astype(BF16)
        mixp[0, 8, 1, :] = mxT[1, 1024].astype(BF16)
        in_maps.append({
            "zp": zb,
            "mixp": mixp,
            "mpc": mpc_bf,
            "mps": mps_bf,
            "wb": wbp,
            "biasb": biasb,
            "edge": edge,
            "jrev": jrev_bf,
        })

    res = run_bass_kernel_spmd(nc, in_maps, core_ids=list(range(N_CORES)))
    out = np.empty((B, HOP * (T - 1)), np.float32)
    for core in range(N_CORES):
        o = res.results[core]["outp"]  # (4, 128, OUTC)
        out[core] = np.ascontiguousarray(np.transpose(o, (2, 0, 1))).reshape(-1)
    return out
